# revision 1
# baseline (speedup 1.0000x reference)
"""Trainium2 Bass kernel for ContextualLoss3D over 8x8x8 patches.

Full inputs x, y: (2, 32, 48, 48, 48) f32. Output: scalar f32 loss.

Strategy: shard the 216 patches across 8 NeuronCores (27 patches each, both
batch elements of a patch on the same core so the y-mean needs no collective).
Each core processes its 54 (n, p) pairs:
  - pack 4 pairs (2 patches x 2 batch) into the 128 SBUF partitions
  - y_mu per (patch, channel) via a free-dim row-sum + one PE matmul that
    combines the two batch halves and broadcasts back to all 4 slots
  - center x/y, channel-norms via squares + block-diagonal-ones PE matmul,
    1/sqrt via ACT log/exp (same table set as the main exp)
  - per pair: gram G = xc^T @ yn on PE (4x (128,512) PSUM chunks),
    row-max on DVE, then w = exp(scale_i*G + bias_i) on ACT with fused
    row-sum accumulation; cx column-max via scaled max-accumulate
    (DVE/GPSIMD), PE transposes, and a free-dim reduce
  - per-core output: sums over its patches of per-patch column-max (2,128,8)
Host: gathers the 8 partial sums, finishes mean over patches, -log, mean.
"""

import numpy as np

import concourse.bass as bass
import concourse.tile as tile
from concourse import mybir
from concourse.bass_utils import run_bass_kernel_spmd

PATCH = 8
N_BATCH = 2
C = 32
M = 512  # 8^3 positions per patch
P_TOT = 216  # (48/8)^3 patches
NCORES = 8
PPC = P_TOT // NCORES  # 27 patches per core
NGROUP = (PPC + 1) // 2  # 14 groups of (2 patches x 2 batch) = 4 pairs
EPS = 1e-5

f32 = mybir.dt.float32
f16 = mybir.dt.float16
AX = mybir.AxisListType.X
OP = mybir.AluOpType
AF = mybir.ActivationFunctionType

_BUILT = None  # cached (nc,) module


def _split_multiwaits(nc):
    """This walrus build supports ONE sync wait per TPB instruction (the 64B
    ISA word has a single events slot). Tile can emit several; split the
    extras into standalone EventSemaphore waits on the same engine, placed
    immediately before the instruction (same sequencer => same semantics)."""
    n_new = 0
    for fn in nc.m.functions:
        for bb in fn.blocks:
            out = []
            for inst in bb.instructions:
                si = inst.sync_info
                if si is not None and si.on_wait and len(si.on_wait) > 1:
                    waits = list(si.on_wait)
                    for w in waits[:-1]:
                        ev = mybir.InstEventSemaphore(
                            name=f"{inst.name}-w{n_new}", ins=[], outs=[]
                        )
                        ev.engine = inst.engine
                        ev.sync_info = mybir.SyncInfo(on_wait=[w], on_update=[])
                        out.append(ev)
                        n_new += 1
                    inst.sync_info = mybir.SyncInfo(
                        on_wait=[waits[-1]], on_update=list(si.on_update)
                    )
                out.append(inst)
            bb.instructions = out
    return n_new


def _pairs_in_group(g):
    # last group has only 1 real patch (27 = 13*2 + 1): pairs q=0 (n=0), q=1 (n=1)
    return 4 if g < NGROUP - 1 else 2


def _build_module():
    nc = bass.Bass(
        "TRN2",
        debug=False,
        enable_asserts=False,
        target_bir_lowering=False,
        num_devices=NCORES,
    )

    X = nc.dram_tensor("xs", [NGROUP, 128, M], f32, kind="ExternalInput").ap()
    Y = nc.dram_tensor("ys", [NGROUP, 128, M], f32, kind="ExternalInput").ap()
    WMU = nc.dram_tensor("wmu", [128, 128], f32, kind="ExternalInput").ap()
    BD = nc.dram_tensor("bd", [128, 4], f32, kind="ExternalInput").ap()
    BDT = nc.dram_tensor("bdt", [4, 128], f32, kind="ExternalInput").ap()
    ID4 = nc.dram_tensor("id4", [4, 4], f32, kind="ExternalInput").ap()
    ID128 = nc.dram_tensor("id128", [128, 128], f16, kind="ExternalInput").ap()
    OUT = nc.dram_tensor("acc_out", [N_BATCH, 128, 8], f32, kind="ExternalOutput").ap()

    with tile.TileContext(nc) as tc:
        with (
            tc.tile_pool(name="consts", bufs=1) as consts,
            tc.tile_pool(name="io", bufs=3) as io,
            tc.tile_pool(name="sb", bufs=3) as sb,
            tc.tile_pool(name="tiny", bufs=6) as tiny,
            tc.tile_pool(name="wpool", bufs=3) as wpool,
            tc.tile_pool(name="accp", bufs=1) as accp,
            tc.tile_pool(name="psA", bufs=2, space="PSUM") as psA,
            tc.tile_pool(name="psB", bufs=4, space="PSUM") as psB,
        ):
            wmu_sb = consts.tile([128, 128], f32, tag="wmu")
            nc.sync.dma_start(wmu_sb, WMU)
            bd_sb = consts.tile([128, 4], f32, tag="bd")
            nc.sync.dma_start(bd_sb, BD)
            bdt_sb = consts.tile([4, 128], f32, tag="bdt")
            nc.sync.dma_start(bdt_sb, BDT)
            id4_sb = consts.tile([4, 4], f32, tag="id4")
            nc.sync.dma_start(id4_sb, ID4)
            id128_sb = consts.tile([128, 128], f16, tag="id128")
            nc.sync.dma_start(id128_sb, ID128)
            c24 = consts.tile([128, 1], f32, tag="c24")
            nc.vector.memset(c24, 1e-24)

            acc = [
                accp.tile([128, 8], f32, tag=f"acc{n}", name=f"acc{n}")
                for n in range(N_BATCH)
            ]
            for a in acc:
                nc.vector.memset(a, 0.0)

            for g in range(NGROUP):
                npair = _pairs_in_group(g)

                xg = io.tile([128, M], f32, tag="xg")
                nc.sync.dma_start(xg, X[g])
                yg = io.tile([128, M], f32, tag="yg")
                nc.sync.dma_start(yg, Y[g])

                # ---- group prep: mean, centering, channel norms ----
                ysum = tiny.tile([128, 1], f32, tag="ysum")
                nc.vector.reduce_sum(ysum, yg, axis=AX)
                mu_ps = psB.tile([128, 1], f32, tag="psB")
                nc.tensor.matmul(mu_ps, wmu_sb, ysum)
                mu = tiny.tile([128, 1], f32, tag="mu")
                nc.vector.tensor_copy(mu, mu_ps)

                xc = sb.tile([128, M], f32, tag="xc")
                nc.vector.tensor_scalar(xc, xg, mu, None, op0=OP.subtract)
                yc = sb.tile([128, M], f32, tag="yc")
                nc.vector.tensor_scalar(yc, yg, mu, None, op0=OP.subtract)

                xsq = sb.tile([128, M], f32, tag="xsq")
                nc.gpsimd.tensor_mul(xsq, xc, xc)
                ysq = sb.tile([128, M], f32, tag="ysq")
                nc.gpsimd.tensor_mul(ysq, yc, yc)

                sx_ps = psB.tile([4, M], f32, tag="psB")
                nc.tensor.matmul(sx_ps, bd_sb, xsq)
                sy_ps = psB.tile([4, M], f32, tag="psB")
                nc.tensor.matmul(sy_ps, bd_sb, ysq)

                # rinv = (S + 1e-24)^-0.5 via log/exp (same ACT table set as Exp)
                ls = sb.tile([4, 2, M], f32, tag="ls")
                nc.scalar.activation(ls[:, 0, :], sx_ps, AF.Ln, bias=c24[:4])
                nc.scalar.activation(ls[:, 1, :], sy_ps, AF.Ln, bias=c24[:4])
                rinv = sb.tile([4, 2, M], f32, tag="rinv")
                nc.scalar.activation(rinv, ls, AF.Exp, scale=-0.5)

                # broadcast y-norms to all 4 slot blocks; yn = yc * rinv_y
                rny_ps = psB.tile([128, M], f32, tag="psB")
                nc.tensor.matmul(rny_ps, bdt_sb, rinv[:, 1, :])
                yn = sb.tile([128, M], f32, tag="yn")
                nc.vector.tensor_mul(yn, yc, rny_ps)

                # x-norms transposed to per-partition layout: invxT[i', c, q]
                invxT_ps = psB.tile([128, 4, 4], f32, tag="psB")
                for c in range(4):
                    nc.tensor.transpose(
                        invxT_ps[:, c, :], rinv[:, 0, 128 * c : 128 * (c + 1)], id4_sb
                    )
                invxT = tiny.tile([128, 4, 4], f32, tag="invxT")
                nc.vector.tensor_copy(invxT, invxT_ps)

                colmax = tiny.tile([128, 4, 4], f32, tag="colmax")

                # ---- per (n, p) pair ----
                for q in range(npair):
                    lo = 32 * q
                    tp = (lo, 0) if lo else None

                    w = wpool.tile([128, 4, M], f16, tag="w")
                    mx4 = tiny.tile([128, 4], f32, tag="mx4")
                    rowsum = tiny.tile([128, 4], f32, tag="rowsum")
                    ghalves = []
                    for h in range(2):
                        gh = psA.tile([128, 2, M], f32, tag="G")
                        ghalves.append(gh)
                        for cc in range(2):
                            c = 2 * h + cc
                            nc.tensor.matmul(
                                gh[:, cc, :],
                                xc[lo : lo + 32, 128 * c : 128 * (c + 1)],
                                yn[lo : lo + 32, :],
                                tile_position=tp,
                            )
                        nc.vector.reduce_max(mx4[:, 2 * h : 2 * h + 2], gh, axis=AX)

                    # scale_i = invx/d, bias_i = 1 - 1/d, d = 1 + eps - invx*mx
                    cm4 = tiny.tile([128, 4], f32, tag="cm4")
                    nc.vector.tensor_mul(cm4, mx4, invxT[:, :, q])
                    d4 = tiny.tile([128, 4], f32, tag="d4")
                    nc.vector.tensor_scalar(
                        d4, cm4, -1.0, 1.0 + EPS, op0=OP.mult, op1=OP.add
                    )
                    q4 = tiny.tile([128, 4], f32, tag="q4")
                    nc.vector.reciprocal(q4, d4)
                    scale4 = tiny.tile([128, 4], f32, tag="scale4")
                    nc.vector.tensor_mul(scale4, q4, invxT[:, :, q])
                    bias4 = tiny.tile([128, 4], f32, tag="bias4")
                    nc.vector.tensor_scalar(
                        bias4, q4, -1.0, 1.0, op0=OP.mult, op1=OP.add
                    )

                    for h in range(2):
                        for cc in range(2):
                            c = 2 * h + cc
                            nc.scalar.activation(
                                w[:, c, :],
                                ghalves[h][:, cc, :],
                                AF.Exp,
                                bias=bias4[:, c : c + 1],
                                scale=scale4[:, c : c + 1],
                                accum_out=rowsum[:, c : c + 1],
                            )

                    r4 = tiny.tile([128, 4], f32, tag="r4")
                    nc.vector.reciprocal(r4, rowsum)

                    # column-max accumulate: macc = max_c (w_c * r_c).
                    # Pool has no max ALU op in this walrus, so DVE does the
                    # fused scale+max (STT); each op is fp16 2x mode.
                    macc = wpool.tile([128, M], f16, tag="macc")
                    nc.vector.tensor_scalar(
                        macc, w[:, 0, :], r4[:, 0:1], None, op0=OP.mult
                    )
                    for c in range(1, 4):
                        nc.vector.scalar_tensor_tensor(
                            macc, w[:, c, :], r4[:, c : c + 1], macc,
                            op0=OP.mult, op1=OP.max,
                        )

                    t_ps = psB.tile([128, 4, 128], f16, tag="psB")
                    for t in range(4):
                        nc.tensor.transpose(
                            t_ps[:, t, :], macc[:, 128 * t : 128 * (t + 1)], id128_sb
                        )
                    nc.vector.reduce_max(colmax[:, q, :], t_ps, axis=AX)

                # accumulate per-batch: pair q has n = q % 2, sub-slot q // 2
                if npair == 4:
                    for n in range(2):
                        nc.vector.tensor_add(
                            acc[n].rearrange("p (s t) -> p s t", s=2),
                            acc[n].rearrange("p (s t) -> p s t", s=2),
                            colmax[:, n::2, :],
                        )
                else:
                    for n in range(2):
                        nc.vector.tensor_add(
                            acc[n][:, 0:4], acc[n][:, 0:4], colmax[:, n, :]
                        )

            for n in range(N_BATCH):
                nc.sync.dma_start(OUT[n], acc[n])

    _split_multiwaits(nc)
    return nc


def _to_patches(v):
    n, c, h, w, d = v.shape
    p = PATCH
    v = v.reshape(n, c, h // p, p, w // p, p, d // p, p)
    v = v.transpose(0, 2, 4, 6, 1, 3, 5, 7)
    return np.ascontiguousarray(v.reshape(n, -1, c, p**3))


def _pack_core(vp, k):
    # vp: (2, 216, 32, 512) -> (NGROUP, 128, 512) for core k
    sl = vp[:, PPC * k : PPC * (k + 1)]  # (2, 27, 32, 512)
    pad = np.zeros((N_BATCH, 2 * NGROUP - PPC, C, M), np.float32)
    arr = np.concatenate([sl, pad], axis=1)  # (2, 28, 32, 512)
    arr = arr.reshape(N_BATCH, NGROUP, 2, C, M)  # [n, g, two, c, m]
    arr = arr.transpose(1, 2, 0, 3, 4)  # [g, two, n, c, m]
    return np.ascontiguousarray(arr.reshape(NGROUP, 128, M))


def _consts():
    kk, pp = np.meshgrid(np.arange(128), np.arange(128), indexing="ij")
    wmu = np.where(
        (kk % 32 == pp % 32) & (kk // 64 == pp // 64), 1.0 / (N_BATCH * M), 0.0
    ).astype(np.float32)
    bd = np.zeros((128, 4), np.float32)
    bd[np.arange(128), np.arange(128) // 32] = 1.0
    bdt = np.ascontiguousarray(bd.T)
    id4 = np.eye(4, dtype=np.float32)
    id128 = np.eye(128, dtype=np.float16)
    return dict(wmu=wmu, bd=bd, bdt=bdt, id4=id4, id128=id128)


def kernel(x, y):
    global _BUILT
    x = np.ascontiguousarray(np.asarray(x), dtype=np.float32)
    y = np.ascontiguousarray(np.asarray(y), dtype=np.float32)
    xp = _to_patches(x)
    yp = _to_patches(y)

    if _BUILT is None:
        _BUILT = _build_module()
    nc = _BUILT

    consts = _consts()
    in_maps = [
        dict(xs=_pack_core(xp, k), ys=_pack_core(yp, k), **consts)
        for k in range(NCORES)
    ]
    res = run_bass_kernel_spmd(nc, in_maps, core_ids=list(range(NCORES)))

    tot = np.zeros((N_BATCH, 128, 8), np.float64)
    for r in res.results:
        tot += r["acc_out"].astype(np.float64)
    tot4 = tot.reshape(N_BATCH, 128, 2, 4).sum(axis=2)  # (2, 128, 4) [n, j', t]
    cx_tot = tot4.transpose(0, 2, 1).reshape(N_BATCH, M) / P_TOT  # j = 128*t + j'
    loss = np.mean(-np.log(cx_tot + EPS))
    return np.float32(loss)



# revision 2
# speedup vs baseline: 49.8493x; 49.8493x over previous
"""Optimized Trainium2 Bass kernel for ContextualLoss3D over 8x8x8 patches.

Baseline math/packing, restructured for pipeline depth:
  - fp16 inputs to all large matmuls (PE fp32 is 4 cycles/row; fp16 is 1)
  - psA bufs=3 so pair q+1 grams overlap pair q's exp/macc
  - per-half scalar chains (smalls) so half-0 exps start while half-1 streams
  - smalls + 2 of 4 macc multiplies on the Pool engine (DVE relief)
  - merged x/y norm PSUM tile -> single Ln/Exp pair per group
"""

import numpy as np

import concourse.bass as bass
import concourse.bass_isa as bass_isa
import concourse.tile as tile
from concourse import mybir
from concourse.bass_utils import run_bass_kernel_spmd

PATCH = 8
N_BATCH = 2
C = 32
M = 512
P_TOT = 216
NCORES = 8
PPC = P_TOT // NCORES
NGROUP = (PPC + 1) // 2
EPS = 1e-5

f32 = mybir.dt.float32
f16 = mybir.dt.float16
AX = mybir.AxisListType.X
OP = mybir.AluOpType
AF = mybir.ActivationFunctionType

_BUILT = None


def _split_multiwaits(nc):
    n_new = 0
    for fn in nc.m.functions:
        for bb in fn.blocks:
            out = []
            for inst in bb.instructions:
                si = inst.sync_info
                if si is not None and si.on_wait and len(si.on_wait) > 1:
                    waits = list(si.on_wait)
                    for w in waits[:-1]:
                        ev = mybir.InstEventSemaphore(
                            name=f"{inst.name}-w{n_new}", ins=[], outs=[]
                        )
                        ev.engine = inst.engine
                        ev.sync_info = mybir.SyncInfo(on_wait=[w], on_update=[])
                        out.append(ev)
                        n_new += 1
                    inst.sync_info = mybir.SyncInfo(
                        on_wait=[waits[-1]], on_update=list(si.on_update)
                    )
                out.append(inst)
            bb.instructions = out
    return n_new


def _pairs_in_group(g):
    return 4 if g < NGROUP - 1 else 2


DEFAULT_CFG = dict(
    psA_bufs=3,
    half_smalls=True,   # per-half scalar chains on Pool
    macc_pool=True,     # 2 of 4 macc multiplies on Pool
    ysum_pool=False,    # Pool tensor_reduce can't do free-axis reduces
    parred=False,       # partition_all_reduce fails walrus codegen (ISA len)
    acc_pool=False,
    stream_macc=True,   # per-half recip + macc ops right after each half's exps
)


def _build_module(cfg=None):
    cfg = dict(DEFAULT_CFG, **(cfg or {}))

    nc = bass.Bass(
        "TRN2",
        debug=False,
        enable_asserts=False,
        target_bir_lowering=False,
        num_devices=NCORES,
    )

    X = nc.dram_tensor("xs", [NGROUP, 128, M], f32, kind="ExternalInput").ap()
    Y = nc.dram_tensor("ys", [NGROUP, 128, M], f32, kind="ExternalInput").ap()
    WMU = nc.dram_tensor("wmu", [128, 128], f32, kind="ExternalInput").ap()
    BD = nc.dram_tensor("bd", [128, 4], f16, kind="ExternalInput").ap()
    BDT = nc.dram_tensor("bdt", [4, 128], f16, kind="ExternalInput").ap()
    ID4 = nc.dram_tensor("id4", [4, 4], f16, kind="ExternalInput").ap()
    ID128 = nc.dram_tensor("id128", [128, 128], f16, kind="ExternalInput").ap()
    if cfg["parred"]:
        OUT = nc.dram_tensor(
            "acc_out", [N_BATCH, 1, M], f16, kind="ExternalOutput"
        ).ap()
    else:
        OUT = nc.dram_tensor(
            "acc_out", [N_BATCH, 128, 8], f32, kind="ExternalOutput"
        ).ap()

    with tile.TileContext(nc) as tc:
        with (
            tc.tile_pool(name="consts", bufs=1) as consts,
            tc.tile_pool(name="io", bufs=3) as io,
            tc.tile_pool(name="sb", bufs=3) as sb,
            tc.tile_pool(name="tiny", bufs=6) as tiny,
            tc.tile_pool(name="wpool", bufs=3) as wpool,
            tc.tile_pool(name="accp", bufs=1) as accp,
            tc.tile_pool(name="psA", bufs=cfg["psA_bufs"], space="PSUM") as psA,
            tc.tile_pool(name="psS", bufs=1, space="PSUM") as psS,
            tc.tile_pool(name="psT", bufs=1, space="PSUM") as psT,
        ):
            wmu_sb = consts.tile([128, 128], f32, tag="wmu")
            nc.sync.dma_start(wmu_sb, WMU)
            bd_sb = consts.tile([128, 4], f16, tag="bd")
            nc.sync.dma_start(bd_sb, BD)
            bdt_sb = consts.tile([4, 128], f16, tag="bdt")
            nc.sync.dma_start(bdt_sb, BDT)
            id4_sb = consts.tile([4, 4], f16, tag="id4")
            nc.sync.dma_start(id4_sb, ID4)
            id128_sb = consts.tile([128, 128], f16, tag="id128")
            nc.sync.dma_start(id128_sb, ID128)
            # norm-squared floor: keeps rinv = (s+floor)^-0.5 <= 1e3 so the
            # padded slots' rinv stays finite in fp16 (else inf*0 => NaN in
            # the PE broadcast matmuls); real slots have s >= O(1).
            c24 = consts.tile([128, 1], f32, tag="c24")
            nc.vector.memset(c24, 1e-6)

            if cfg["parred"]:
                acc = [
                    accp.tile([128, M], f16, tag=f"acc{n}", name=f"acc{n}")
                    for n in range(N_BATCH)
                ]
            else:
                acc = [
                    accp.tile([128, 8], f32, tag=f"acc{n}", name=f"acc{n}")
                    for n in range(N_BATCH)
                ]
            for a in acc:
                nc.vector.memset(a, 0.0)

            for g in range(NGROUP):
                npair = _pairs_in_group(g)

                xg = io.tile([128, M], f32, tag="xg")
                nc.sync.dma_start(xg, X[g])
                yg = io.tile([128, M], f32, tag="yg")
                nc.sync.dma_start(yg, Y[g])

                # ---- group prep: mean, centering, channel norms ----
                ysum = tiny.tile([128, 1], f32, tag="ysum")
                if cfg["ysum_pool"]:
                    nc.gpsimd.reduce_sum(ysum, yg, axis=AX)
                else:
                    nc.vector.reduce_sum(ysum, yg, axis=AX)
                mu_ps = psS.tile([128, 1], f32, tag="psS")
                nc.tensor.matmul(mu_ps, wmu_sb, ysum)
                mu = tiny.tile([128, 1], f32, tag="mu")
                nc.vector.tensor_copy(mu, mu_ps)

                xc = sb.tile([128, M], f16, tag="xc")
                nc.vector.tensor_scalar(xc, xg, mu, None, op0=OP.subtract)
                yc = sb.tile([128, M], f16, tag="yc")
                nc.vector.tensor_scalar(yc, yg, mu, None, op0=OP.subtract)

                xsq = sb.tile([128, M], f16, tag="xsq")
                nc.gpsimd.tensor_mul(xsq, xc, xc)
                ysq = sb.tile([128, M], f16, tag="ysq")
                nc.gpsimd.tensor_mul(ysq, yc, yc)

                sx_ps = psS.tile([4, M], f32, tag="psS")
                nc.tensor.matmul(sx_ps, bd_sb, xsq)
                sy_ps = psS.tile([4, M], f32, tag="psS")
                nc.tensor.matmul(sy_ps, bd_sb, ysq)

                # rinv = (S + 1e-24)^-0.5 via log/exp (same ACT table set)
                ls = sb.tile([4, 2, M], f32, tag="ls")
                nc.scalar.activation(ls[:, 0, :], sx_ps, AF.Ln, bias=c24[:4])
                nc.scalar.activation(ls[:, 1, :], sy_ps, AF.Ln, bias=c24[:4])
                rinv = sb.tile([4, 2, M], f16, tag="rinv")
                nc.scalar.activation(rinv, ls, AF.Exp, scale=-0.5)

                # broadcast y-norms to all 4 slot blocks; yn = yc * rinv_y
                rny_ps = psS.tile([128, M], f32, tag="psS")
                nc.tensor.matmul(rny_ps, bdt_sb, rinv[:, 1, :])
                yn = sb.tile([128, M], f16, tag="yn")
                nc.vector.tensor_mul(yn, yc, rny_ps)

                # x-norms transposed to per-partition layout: invxT[i', c, q]
                invxT_ps = psS.tile([128, 4, 4], f16, tag="psS")
                for c in range(4):
                    nc.tensor.transpose(
                        invxT_ps[:, c, :], rinv[:, 0, 128 * c : 128 * (c + 1)], id4_sb
                    )
                invxT = tiny.tile([128, 4, 4], f32, tag="invxT")
                nc.vector.tensor_copy(invxT, invxT_ps)

                if not cfg["parred"]:
                    colmax = tiny.tile([128, 4, 4], f32, tag="colmax")

                # ---- per (n, p) pair ----
                for q in range(npair):
                    lo = 32 * q
                    tp = (lo, 0) if lo else None

                    w = wpool.tile([128, 4, M], f16, tag="w")
                    mx4 = tiny.tile([128, 4], f32, tag="mx4")
                    rowsum = tiny.tile([128, 4], f32, tag="rowsum")
                    scale4 = tiny.tile([128, 4], f32, tag="scale4")
                    bias4 = tiny.tile([128, 4], f32, tag="bias4")
                    q4 = tiny.tile([128, 4], f32, tag="q4")
                    cm4 = tiny.tile([128, 4], f32, tag="cm4")
                    d4 = tiny.tile([128, 4], f32, tag="d4")
                    r4 = tiny.tile([128, 4], f32, tag="r4")
                    macc = wpool.tile([128, M], f16, tag="macc")
                    for h in range(2):
                        sl = slice(2 * h, 2 * h + 2)
                        gh = psA.tile([128, 2, M], f32, tag="G")
                        for cc in range(2):
                            c = 2 * h + cc
                            nc.tensor.matmul(
                                gh[:, cc, :],
                                xc[lo : lo + 32, 128 * c : 128 * (c + 1)],
                                yn[lo : lo + 32, :],
                                tile_position=tp,
                            )
                        nc.vector.reduce_max(mx4[:, sl], gh, axis=AX)
                        # scale_i = invx/d, bias_i = 1-1/d, d = 1+eps-invx*mx
                        nc.gpsimd.tensor_mul(
                            cm4[:, sl], mx4[:, sl], invxT[:, sl, q]
                        )
                        nc.gpsimd.tensor_scalar(
                            d4[:, sl], cm4[:, sl], -1.0, 1.0 + EPS,
                            op0=OP.mult, op1=OP.add,
                        )
                        nc.vector.reciprocal(q4[:, sl], d4[:, sl])
                        nc.gpsimd.tensor_mul(
                            scale4[:, sl], q4[:, sl], invxT[:, sl, q]
                        )
                        nc.gpsimd.tensor_scalar(
                            bias4[:, sl], q4[:, sl], -1.0, 1.0,
                            op0=OP.mult, op1=OP.add,
                        )
                        for cc in range(2):
                            c = 2 * h + cc
                            nc.scalar.activation(
                                w[:, c, :],
                                gh[:, cc, :],
                                AF.Exp,
                                bias=bias4[:, c : c + 1],
                                scale=scale4[:, c : c + 1],
                                accum_out=rowsum[:, c : c + 1],
                            )
                        nc.vector.reciprocal(r4[:, sl], rowsum[:, sl])
                        # column-max accumulate: macc = max_c (w_c * r_c),
                        # streamed per half (half 1's mults on Pool)
                        if h == 0:
                            nc.vector.tensor_scalar(
                                macc, w[:, 0, :], r4[:, 0:1], None, op0=OP.mult
                            )
                            nc.vector.scalar_tensor_tensor(
                                macc, w[:, 1, :], r4[:, 1:2], macc,
                                op0=OP.mult, op1=OP.max,
                            )
                        else:
                            tmp2 = wpool.tile([128, M], f16, tag="tmp2")
                            nc.gpsimd.tensor_scalar(
                                tmp2, w[:, 2, :], r4[:, 2:3], None, op0=OP.mult
                            )
                            tmp3 = wpool.tile([128, M], f16, tag="tmp3")
                            nc.gpsimd.tensor_scalar(
                                tmp3, w[:, 3, :], r4[:, 3:4], None, op0=OP.mult
                            )
                            m23 = wpool.tile([128, M], f16, tag="m23")
                            nc.vector.tensor_tensor(m23, tmp2, tmp3, op=OP.max)
                            nc.vector.tensor_tensor(macc, macc, m23, op=OP.max)

                    if cfg["parred"]:
                        # column max over all 512 i at once: Pool all-reduce
                        # across partitions (result replicated on 128 rows)
                        par = wpool.tile([128, M], f16, tag="par")
                        nc.gpsimd.partition_all_reduce(
                            par, macc, channels=128,
                            reduce_op=bass_isa.ReduceOp.max,
                        )
                        if cfg["acc_pool"]:
                            nc.gpsimd.tensor_add(acc[q % 2], acc[q % 2], par)
                        else:
                            nc.vector.tensor_add(acc[q % 2], acc[q % 2], par)
                    else:
                        t_ps = psT.tile([128, 4, 128], f16, tag="psT")
                        for t in range(4):
                            nc.tensor.transpose(
                                t_ps[:, t, :], macc[:, 128 * t : 128 * (t + 1)],
                                id128_sb,
                            )
                        nc.vector.reduce_max(colmax[:, q, :], t_ps, axis=AX)

                # accumulate per-batch: pair q has n = q % 2, sub-slot q // 2
                if not cfg["parred"]:
                    if npair == 4:
                        for n in range(2):
                            nc.vector.tensor_add(
                                acc[n].rearrange("p (s t) -> p s t", s=2),
                                acc[n].rearrange("p (s t) -> p s t", s=2),
                                colmax[:, n::2, :],
                            )
                    else:
                        for n in range(2):
                            nc.vector.tensor_add(
                                acc[n][:, 0:4], acc[n][:, 0:4], colmax[:, n, :]
                            )

            for n in range(N_BATCH):
                if cfg["parred"]:
                    nc.sync.dma_start(OUT[n], acc[n][0:1, :])
                else:
                    nc.sync.dma_start(OUT[n], acc[n])

    _split_multiwaits(nc)
    return nc


def _to_patches(v):
    n, c, h, w, d = v.shape
    p = PATCH
    v = v.reshape(n, c, h // p, p, w // p, p, d // p, p)
    v = v.transpose(0, 2, 4, 6, 1, 3, 5, 7)
    return np.ascontiguousarray(v.reshape(n, -1, c, p**3))


def _pack_core(vp, k):
    sl = vp[:, PPC * k : PPC * (k + 1)]
    pad = np.zeros((N_BATCH, 2 * NGROUP - PPC, C, M), np.float32)
    arr = np.concatenate([sl, pad], axis=1)
    arr = arr.reshape(N_BATCH, NGROUP, 2, C, M)
    arr = arr.transpose(1, 2, 0, 3, 4)
    return np.ascontiguousarray(arr.reshape(NGROUP, 128, M))


def _consts():
    kk, pp = np.meshgrid(np.arange(128), np.arange(128), indexing="ij")
    wmu = np.where(
        (kk % 32 == pp % 32) & (kk // 64 == pp // 64), 1.0 / (N_BATCH * M), 0.0
    ).astype(np.float32)
    bd = np.zeros((128, 4), np.float16)
    bd[np.arange(128), np.arange(128) // 32] = 1.0
    bdt = np.ascontiguousarray(bd.T)
    id4 = np.eye(4, dtype=np.float16)
    id128 = np.eye(128, dtype=np.float16)
    return dict(wmu=wmu, bd=bd, bdt=bdt, id4=id4, id128=id128)


def kernel(x, y):
    global _BUILT
    x = np.ascontiguousarray(np.asarray(x), dtype=np.float32)
    y = np.ascontiguousarray(np.asarray(y), dtype=np.float32)
    xp = _to_patches(x)
    yp = _to_patches(y)

    if _BUILT is None:
        _BUILT = _build_module()
    nc = _BUILT

    consts = _consts()
    in_maps = [
        dict(xs=_pack_core(xp, k), ys=_pack_core(yp, k), **consts)
        for k in range(NCORES)
    ]
    res = run_bass_kernel_spmd(nc, in_maps, core_ids=list(range(NCORES)))

    if DEFAULT_CFG["parred"]:
        tot = np.zeros((N_BATCH, M), np.float64)
        for r in res.results:
            tot += r["acc_out"].astype(np.float64).reshape(N_BATCH, M)
        cx_tot = tot / P_TOT
    else:
        tot = np.zeros((N_BATCH, 128, 8), np.float64)
        for r in res.results:
            tot += r["acc_out"].astype(np.float64)
        tot4 = tot.reshape(N_BATCH, 128, 2, 4).sum(axis=2)
        cx_tot = tot4.transpose(0, 2, 1).reshape(N_BATCH, M) / P_TOT
    loss = np.mean(-np.log(cx_tot + EPS))
    return np.float32(loss)


# revision 10
# speedup vs baseline: 58.5874x; 1.1753x over previous
"""Optimized Trainium2 Bass kernel for ContextualLoss3D over 8x8x8 patches.

Baseline math/packing, restructured for pipeline depth:
  - fp16 inputs to all large matmuls (PE fp32 is 4 cycles/row; fp16 is 1)
  - psA bufs=3 so pair q+1 grams overlap pair q's exp/macc
  - per-half scalar chains (smalls) so half-0 exps start while half-1 streams
  - smalls + 2 of 4 macc multiplies on the Pool engine (DVE relief)
  - merged x/y norm PSUM tile -> single Ln/Exp pair per group
"""

import numpy as np

import concourse.bass as bass
import concourse.bass_isa as bass_isa
import concourse.tile as tile
from concourse import mybir
from concourse.bass_utils import run_bass_kernel_spmd

PATCH = 8
N_BATCH = 2
C = 32
M = 512
P_TOT = 216
NCORES = 8
PPC = P_TOT // NCORES
NGROUP = (PPC + 1) // 2
EPS = 1e-5

f32 = mybir.dt.float32
f16 = mybir.dt.float16
AX = mybir.AxisListType.X
OP = mybir.AluOpType
AF = mybir.ActivationFunctionType

_BUILT = None


def _split_multiwaits(nc):
    n_new = 0
    for fn in nc.m.functions:
        for bb in fn.blocks:
            out = []
            for inst in bb.instructions:
                si = inst.sync_info
                if si is not None and si.on_wait and len(si.on_wait) > 1:
                    waits = list(si.on_wait)
                    for w in waits[:-1]:
                        ev = mybir.InstEventSemaphore(
                            name=f"{inst.name}-w{n_new}", ins=[], outs=[]
                        )
                        ev.engine = inst.engine
                        ev.sync_info = mybir.SyncInfo(on_wait=[w], on_update=[])
                        out.append(ev)
                        n_new += 1
                    inst.sync_info = mybir.SyncInfo(
                        on_wait=[waits[-1]], on_update=list(si.on_update)
                    )
                out.append(inst)
            bb.instructions = out
    return n_new


def _pairs_in_group(g):
    return 4 if g < NGROUP - 1 else 2


DEFAULT_CFG = dict(
    psA_bufs=3,
    half_smalls=True,   # per-half scalar chains on Pool
    macc_pool=True,     # 2 of 4 macc multiplies on Pool
    ysum_pool=False,    # Pool tensor_reduce can't do free-axis reduces
    parred=False,       # partition_all_reduce fails walrus codegen (ISA len)
    acc_pool=False,
    stream_macc=True,   # per-half recip + macc ops right after each half's exps
    smalls_eng="pool",  # engine for the per-half scalar chains
    maccmul_eng="pool", # engine for macc half-1 multiplies
)


def _build_module(cfg=None):
    cfg = dict(DEFAULT_CFG, **(cfg or {}))

    nc = bass.Bass(
        "TRN2",
        debug=False,
        enable_asserts=False,
        target_bir_lowering=False,
        num_devices=NCORES,
    )

    X = nc.dram_tensor("xs", [NGROUP, 128, M], f32, kind="ExternalInput").ap()
    Y = nc.dram_tensor("ys", [NGROUP, 128, M], f32, kind="ExternalInput").ap()
    WMU = nc.dram_tensor("wmu", [128, 128], f32, kind="ExternalInput").ap()
    BD = nc.dram_tensor("bd", [128, 4], f16, kind="ExternalInput").ap()
    BDT = nc.dram_tensor("bdt", [4, 128], f16, kind="ExternalInput").ap()
    ID4 = nc.dram_tensor("id4", [4, 4], f16, kind="ExternalInput").ap()
    ID128 = nc.dram_tensor("id128", [128, 128], f16, kind="ExternalInput").ap()
    if cfg["parred"]:
        OUT = nc.dram_tensor(
            "acc_out", [N_BATCH, 1, M], f16, kind="ExternalOutput"
        ).ap()
    else:
        OUT = nc.dram_tensor(
            "acc_out", [N_BATCH, 128, 8], f32, kind="ExternalOutput"
        ).ap()

    with tile.TileContext(nc) as tc:
        with (
            tc.tile_pool(name="consts", bufs=1) as consts,
            tc.tile_pool(name="io", bufs=3) as io,
            tc.tile_pool(name="sb", bufs=3) as sb,
            tc.tile_pool(name="tiny", bufs=6) as tiny,
            tc.tile_pool(name="wpool", bufs=3) as wpool,
            tc.tile_pool(name="accp", bufs=1) as accp,
            tc.tile_pool(name="psA", bufs=cfg["psA_bufs"], space="PSUM") as psA,
            tc.tile_pool(name="psS", bufs=1, space="PSUM") as psS,
            tc.tile_pool(name="psT", bufs=1, space="PSUM") as psT,
        ):
            wmu_sb = consts.tile([128, 128], f32, tag="wmu")
            nc.sync.dma_start(wmu_sb, WMU)
            bd_sb = consts.tile([128, 4], f16, tag="bd")
            nc.sync.dma_start(bd_sb, BD)
            bdt_sb = consts.tile([4, 128], f16, tag="bdt")
            nc.sync.dma_start(bdt_sb, BDT)
            id4_sb = consts.tile([4, 4], f16, tag="id4")
            nc.sync.dma_start(id4_sb, ID4)
            id128_sb = consts.tile([128, 128], f16, tag="id128")
            nc.sync.dma_start(id128_sb, ID128)
            # norm-squared floor: keeps rinv = (s+floor)^-0.5 <= 1e3 so the
            # padded slots' rinv stays finite in fp16 (else inf*0 => NaN in
            # the PE broadcast matmuls); real slots have s >= O(1).
            c24 = consts.tile([128, 1], f32, tag="c24")
            nc.vector.memset(c24, 1e-6)

            if cfg["parred"]:
                acc = [
                    accp.tile([128, M], f16, tag=f"acc{n}", name=f"acc{n}")
                    for n in range(N_BATCH)
                ]
            else:
                acc = [
                    accp.tile([128, 8], f32, tag=f"acc{n}", name=f"acc{n}")
                    for n in range(N_BATCH)
                ]
            for a in acc:
                nc.vector.memset(a, 0.0)

            for g in range(NGROUP):
                npair = _pairs_in_group(g)

                xg = io.tile([128, M], f32, tag="xg")
                nc.sync.dma_start(xg, X[g])
                yg = io.tile([128, M], f32, tag="yg")
                nc.sync.dma_start(yg, Y[g])

                # ---- group prep: mean, centering, channel norms ----
                ysum = tiny.tile([128, 1], f32, tag="ysum")
                if cfg["ysum_pool"]:
                    nc.gpsimd.reduce_sum(ysum, yg, axis=AX)
                else:
                    nc.vector.reduce_sum(ysum, yg, axis=AX)
                mu_ps = psS.tile([128, 1], f32, tag="psS")
                nc.tensor.matmul(mu_ps, wmu_sb, ysum)
                mu = tiny.tile([128, 1], f32, tag="mu")
                nc.vector.tensor_copy(mu, mu_ps)

                xc = sb.tile([128, M], f16, tag="xc")
                nc.vector.tensor_scalar(xc, xg, mu, None, op0=OP.subtract)
                yc = sb.tile([128, M], f16, tag="yc")
                nc.vector.tensor_scalar(yc, yg, mu, None, op0=OP.subtract)

                xsq = sb.tile([128, M], f16, tag="xsq")
                nc.gpsimd.tensor_mul(xsq, xc, xc)
                ysq = sb.tile([128, M], f16, tag="ysq")
                nc.gpsimd.tensor_mul(ysq, yc, yc)

                sx_ps = psS.tile([4, M], f32, tag="psS")
                nc.tensor.matmul(sx_ps, bd_sb, xsq)
                sy_ps = psS.tile([4, M], f32, tag="psS")
                nc.tensor.matmul(sy_ps, bd_sb, ysq)

                # rinv = (S + 1e-24)^-0.5 via log/exp (same ACT table set)
                ls = sb.tile([4, 2, M], f32, tag="ls")
                nc.scalar.activation(ls[:, 0, :], sx_ps, AF.Ln, bias=c24[:4])
                nc.scalar.activation(ls[:, 1, :], sy_ps, AF.Ln, bias=c24[:4])
                rinv = sb.tile([4, 2, M], f16, tag="rinv")
                nc.scalar.activation(rinv, ls, AF.Exp, scale=-0.5)

                # broadcast y-norms to all 4 slot blocks; yn = yc * rinv_y
                rny_ps = psS.tile([128, M], f32, tag="psS")
                nc.tensor.matmul(rny_ps, bdt_sb, rinv[:, 1, :])
                yn = sb.tile([128, M], f16, tag="yn")
                nc.vector.tensor_mul(yn, yc, rny_ps)

                # x-norms transposed to per-partition layout: invxT[i', c, q]
                invxT_ps = psS.tile([128, 4, 4], f16, tag="psS")
                for c in range(4):
                    nc.tensor.transpose(
                        invxT_ps[:, c, :], rinv[:, 0, 128 * c : 128 * (c + 1)], id4_sb
                    )
                invxT = tiny.tile([128, 4, 4], f32, tag="invxT")
                nc.vector.tensor_copy(invxT, invxT_ps)

                if not cfg["parred"] and cfg.get("ablate", 0) < 3:
                    colmax = tiny.tile([128, 4, 4], f32, tag="colmax")

                # ---- per (n, p) pair ----
                for q in range(npair):
                    lo = 32 * q
                    tp = (lo, 0) if lo else None

                    w = wpool.tile([128, 4, M], f16, tag="w")
                    mx4 = tiny.tile([128, 4], f32, tag="mx4")
                    rowsum = tiny.tile([128, 4], f32, tag="rowsum")
                    scale4 = tiny.tile([128, 4], f32, tag="scale4")
                    bias4 = tiny.tile([128, 4], f32, tag="bias4")
                    q4 = tiny.tile([128, 4], f32, tag="q4")
                    cm4 = tiny.tile([128, 4], f32, tag="cm4")
                    d4 = tiny.tile([128, 4], f32, tag="d4")
                    r4 = tiny.tile([128, 4], f32, tag="r4")
                    macc = wpool.tile([128, M], f16, tag="macc")
                    for h in range(2):
                        sl = slice(2 * h, 2 * h + 2)
                        gh = psA.tile([128, 2, M], f32, tag="G")
                        for cc in range(2):
                            c = 2 * h + cc
                            nc.tensor.matmul(
                                gh[:, cc, :],
                                xc[lo : lo + 32, 128 * c : 128 * (c + 1)],
                                yn[lo : lo + 32, :],
                                tile_position=tp,
                            )
                        abl = cfg.get("ablate", 0)
                        if abl < 2:
                            nc.vector.reduce_max(mx4[:, sl], gh, axis=AX)
                            # scale_i = invx/d, bias_i = 1-1/d,
                            # d = 1+eps-invx*mx
                            sme = (nc.gpsimd if cfg["smalls_eng"] == "pool"
                                   else nc.vector)
                            sme.tensor_mul(
                                cm4[:, sl], mx4[:, sl], invxT[:, sl, q]
                            )
                            sme.tensor_scalar(
                                d4[:, sl], cm4[:, sl], -1.0, 1.0 + EPS,
                                op0=OP.mult, op1=OP.add,
                            )
                            nc.vector.reciprocal(q4[:, sl], d4[:, sl])
                            sme.tensor_mul(
                                scale4[:, sl], q4[:, sl], invxT[:, sl, q]
                            )
                            sme.tensor_scalar(
                                bias4[:, sl], q4[:, sl], -1.0, 1.0,
                                op0=OP.mult, op1=OP.add,
                            )
                        if abl < 4:
                            for cc in range(2):
                                c = 2 * h + cc
                                kw = (dict(bias=bias4[:, c : c + 1],
                                           scale=scale4[:, c : c + 1])
                                      if abl < 2 else dict(scale=0.01))
                                nc.scalar.activation(
                                    w[:, c, :],
                                    gh[:, cc, :],
                                    AF.Exp,
                                    accum_out=rowsum[:, c : c + 1],
                                    **kw,
                                )
                            nc.vector.reciprocal(r4[:, sl], rowsum[:, sl])
                        if abl >= 3:
                            nc.vector.tensor_add(
                                acc[q % 2][:, 0:4], acc[q % 2][:, 0:4],
                                gh[:, 0, 0:4],
                            )
                            continue
                        # column-max accumulate: macc = max_c (w_c * r_c),
                        # streamed per half (half 1's mults on Pool)
                        if h == 0:
                            nc.vector.tensor_scalar(
                                macc, w[:, 0, :], r4[:, 0:1], None, op0=OP.mult
                            )
                            nc.vector.scalar_tensor_tensor(
                                macc, w[:, 1, :], r4[:, 1:2], macc,
                                op0=OP.mult, op1=OP.max,
                            )
                        elif cfg["maccmul_eng"] == "pool":
                            tmp2 = wpool.tile([128, M], f16, tag="tmp2")
                            nc.gpsimd.tensor_scalar(
                                tmp2, w[:, 2, :], r4[:, 2:3], None, op0=OP.mult
                            )
                            tmp3 = wpool.tile([128, M], f16, tag="tmp3")
                            nc.gpsimd.tensor_scalar(
                                tmp3, w[:, 3, :], r4[:, 3:4], None, op0=OP.mult
                            )
                            m23 = wpool.tile([128, M], f16, tag="m23")
                            nc.vector.tensor_tensor(m23, tmp2, tmp3, op=OP.max)
                            nc.vector.tensor_tensor(macc, macc, m23, op=OP.max)
                        else:
                            for cc in range(2):
                                c = 2 + cc
                                nc.vector.scalar_tensor_tensor(
                                    macc, w[:, c, :], r4[:, c : c + 1], macc,
                                    op0=OP.mult, op1=OP.max,
                                )

                    if cfg.get("ablate", 0) >= 3:
                        pass
                    elif cfg["parred"]:
                        # column max over all 512 i at once: Pool all-reduce
                        # across partitions (result replicated on 128 rows)
                        par = wpool.tile([128, M], f16, tag="par")
                        nc.gpsimd.partition_all_reduce(
                            par, macc, channels=128,
                            reduce_op=bass_isa.ReduceOp.max,
                        )
                        if cfg["acc_pool"]:
                            nc.gpsimd.tensor_add(acc[q % 2], acc[q % 2], par)
                        else:
                            nc.vector.tensor_add(acc[q % 2], acc[q % 2], par)
                    else:
                        t_ps = psT.tile([128, 4, 128], f16, tag="psT")
                        for t in range(4):
                            nc.tensor.transpose(
                                t_ps[:, t, :], macc[:, 128 * t : 128 * (t + 1)],
                                id128_sb,
                            )
                        nc.vector.reduce_max(colmax[:, q, :], t_ps, axis=AX)

                # accumulate per-batch: pair q has n = q % 2, sub-slot q // 2
                if not cfg["parred"] and cfg.get("ablate", 0) < 3:
                    if npair == 4:
                        for n in range(2):
                            nc.vector.tensor_add(
                                acc[n].rearrange("p (s t) -> p s t", s=2),
                                acc[n].rearrange("p (s t) -> p s t", s=2),
                                colmax[:, n::2, :],
                            )
                    else:
                        for n in range(2):
                            nc.vector.tensor_add(
                                acc[n][:, 0:4], acc[n][:, 0:4], colmax[:, n, :]
                            )

            for n in range(N_BATCH):
                if cfg["parred"]:
                    nc.sync.dma_start(OUT[n], acc[n][0:1, :])
                else:
                    nc.sync.dma_start(OUT[n], acc[n])

    _split_multiwaits(nc)
    return nc


def _to_patches(v):
    n, c, h, w, d = v.shape
    p = PATCH
    v = v.reshape(n, c, h // p, p, w // p, p, d // p, p)
    v = v.transpose(0, 2, 4, 6, 1, 3, 5, 7)
    return np.ascontiguousarray(v.reshape(n, -1, c, p**3))


def _pack_core(vp, k):
    sl = vp[:, PPC * k : PPC * (k + 1)]
    pad = np.zeros((N_BATCH, 2 * NGROUP - PPC, C, M), np.float32)
    arr = np.concatenate([sl, pad], axis=1)
    arr = arr.reshape(N_BATCH, NGROUP, 2, C, M)
    arr = arr.transpose(1, 2, 0, 3, 4)
    return np.ascontiguousarray(arr.reshape(NGROUP, 128, M))


def _consts():
    kk, pp = np.meshgrid(np.arange(128), np.arange(128), indexing="ij")
    wmu = np.where(
        (kk % 32 == pp % 32) & (kk // 64 == pp // 64), 1.0 / (N_BATCH * M), 0.0
    ).astype(np.float32)
    bd = np.zeros((128, 4), np.float16)
    bd[np.arange(128), np.arange(128) // 32] = 1.0
    bdt = np.ascontiguousarray(bd.T)
    id4 = np.eye(4, dtype=np.float16)
    id128 = np.eye(128, dtype=np.float16)
    return dict(wmu=wmu, bd=bd, bdt=bdt, id4=id4, id128=id128)


def kernel(x, y):
    global _BUILT
    x = np.ascontiguousarray(np.asarray(x), dtype=np.float32)
    y = np.ascontiguousarray(np.asarray(y), dtype=np.float32)
    xp = _to_patches(x)
    yp = _to_patches(y)

    if _BUILT is None:
        _BUILT = _build_module()
    nc = _BUILT

    consts = _consts()
    in_maps = [
        dict(xs=_pack_core(xp, k), ys=_pack_core(yp, k), **consts)
        for k in range(NCORES)
    ]
    res = run_bass_kernel_spmd(nc, in_maps, core_ids=list(range(NCORES)))

    if DEFAULT_CFG["parred"]:
        tot = np.zeros((N_BATCH, M), np.float64)
        for r in res.results:
            tot += r["acc_out"].astype(np.float64).reshape(N_BATCH, M)
        cx_tot = tot / P_TOT
    else:
        tot = np.zeros((N_BATCH, 128, 8), np.float64)
        for r in res.results:
            tot += r["acc_out"].astype(np.float64)
        tot4 = tot.reshape(N_BATCH, 128, 2, 4).sum(axis=2)
        cx_tot = tot4.transpose(0, 2, 1).reshape(N_BATCH, M) / P_TOT
    loss = np.mean(-np.log(cx_tot + EPS))
    return np.float32(loss)


# revision 16
# speedup vs baseline: 59.4502x; 1.0147x over previous
"""Optimized Trainium2 Bass kernel for ContextualLoss3D over 8x8x8 patches.

Baseline math/packing, restructured for pipeline depth:
  - fp16 inputs to all large matmuls (PE fp32 is 4 cycles/row; fp16 is 1)
  - psA bufs=3 so pair q+1 grams overlap pair q's exp/macc
  - per-half scalar chains (smalls) so half-0 exps start while half-1 streams
  - smalls + 2 of 4 macc multiplies on the Pool engine (DVE relief)
  - merged x/y norm PSUM tile -> single Ln/Exp pair per group
"""

import numpy as np

import concourse.bass as bass
import concourse.bass_isa as bass_isa
import concourse.tile as tile
from concourse import mybir
from concourse.bass_utils import run_bass_kernel_spmd

PATCH = 8
N_BATCH = 2
C = 32
M = 512
P_TOT = 216
NCORES = 8
PPC = P_TOT // NCORES
NGROUP = (PPC + 1) // 2
EPS = 1e-5

f32 = mybir.dt.float32
f16 = mybir.dt.float16
AX = mybir.AxisListType.X
OP = mybir.AluOpType
AF = mybir.ActivationFunctionType

_BUILT = None


def _split_multiwaits(nc):
    n_new = 0
    for fn in nc.m.functions:
        for bb in fn.blocks:
            out = []
            for inst in bb.instructions:
                si = inst.sync_info
                if si is not None and si.on_wait and len(si.on_wait) > 1:
                    waits = list(si.on_wait)
                    for w in waits[:-1]:
                        ev = mybir.InstEventSemaphore(
                            name=f"{inst.name}-w{n_new}", ins=[], outs=[]
                        )
                        ev.engine = inst.engine
                        ev.sync_info = mybir.SyncInfo(on_wait=[w], on_update=[])
                        out.append(ev)
                        n_new += 1
                    inst.sync_info = mybir.SyncInfo(
                        on_wait=[waits[-1]], on_update=list(si.on_update)
                    )
                out.append(inst)
            bb.instructions = out
    return n_new


def _pairs_in_group(g):
    return 4 if g < NGROUP - 1 else 2


DEFAULT_CFG = dict(
    psA_bufs=3,
    half_smalls=True,   # per-half scalar chains on Pool
    macc_pool=True,     # 2 of 4 macc multiplies on Pool
    ysum_pool=False,    # Pool tensor_reduce can't do free-axis reduces
    parred=False,       # partition_all_reduce fails walrus codegen (ISA len)
    acc_pool=False,
    stream_macc=True,   # per-half recip + macc ops right after each half's exps
    smalls_eng="pool",  # engine for the per-half scalar chains
    maccmul_eng="pool", # engine for macc half-1 multiplies
    ysum_act=True,      # group row-sum via ACT Copy accumulator (DVE relief)
    center_pool=False,  # Pool centering adds latency on the group chain
    xnorm_direct=True,  # x-norms via 16 tiny PE matmuls into [i',c,q] layout
)


def _build_module(cfg=None):
    cfg = dict(DEFAULT_CFG, **(cfg or {}))

    nc = bass.Bass(
        "TRN2",
        debug=False,
        enable_asserts=False,
        target_bir_lowering=False,
        num_devices=NCORES,
    )

    X = nc.dram_tensor("xs", [NGROUP, 128, M], f32, kind="ExternalInput").ap()
    Y = nc.dram_tensor("ys", [NGROUP, 128, M], f32, kind="ExternalInput").ap()
    WMU = nc.dram_tensor("wmu", [128, 128], f32, kind="ExternalInput").ap()
    BD = nc.dram_tensor("bd", [128, 4], f16, kind="ExternalInput").ap()
    BDT = nc.dram_tensor("bdt", [4, 128], f16, kind="ExternalInput").ap()
    ID4 = nc.dram_tensor("id4", [4, 4], f16, kind="ExternalInput").ap()
    ID128 = nc.dram_tensor("id128", [128, 128], f16, kind="ExternalInput").ap()
    if cfg["parred"]:
        OUT = nc.dram_tensor(
            "acc_out", [N_BATCH, 1, M], f16, kind="ExternalOutput"
        ).ap()
    else:
        OUT = nc.dram_tensor(
            "acc_out", [N_BATCH, 128, 8], f32, kind="ExternalOutput"
        ).ap()

    with tile.TileContext(nc) as tc:
        with (
            tc.tile_pool(name="consts", bufs=1) as consts,
            tc.tile_pool(name="io", bufs=3) as io,
            tc.tile_pool(name="sb", bufs=3) as sb,
            tc.tile_pool(name="tiny", bufs=6) as tiny,
            tc.tile_pool(name="wpool", bufs=3) as wpool,
            tc.tile_pool(name="accp", bufs=1) as accp,
            tc.tile_pool(name="psA", bufs=cfg["psA_bufs"], space="PSUM") as psA,
            tc.tile_pool(name="psS", bufs=1, space="PSUM") as psS,
            tc.tile_pool(name="psT", bufs=cfg.get("psT_bufs", 1), space="PSUM") as psT,
        ):
            wmu_sb = consts.tile([128, 128], f32, tag="wmu")
            nc.sync.dma_start(wmu_sb, WMU)
            bd_sb = consts.tile([128, 4], f16, tag="bd")
            nc.sync.dma_start(bd_sb, BD)
            bdt_sb = consts.tile([4, 128], f16, tag="bdt")
            nc.sync.dma_start(bdt_sb, BDT)
            id4_sb = consts.tile([4, 4], f16, tag="id4")
            nc.sync.dma_start(id4_sb, ID4)
            id128_sb = consts.tile([128, 128], f16, tag="id128")
            nc.sync.dma_start(id128_sb, ID128)
            # norm-squared floor: keeps rinv = (s+floor)^-0.5 <= 1e3 so the
            # padded slots' rinv stays finite in fp16 (else inf*0 => NaN in
            # the PE broadcast matmuls); real slots have s >= O(1).
            c24 = consts.tile([128, 1], f32, tag="c24")
            nc.vector.memset(c24, 1e-6)
            ones32 = consts.tile([128, 1], f16, tag="ones32")
            nc.vector.memset(ones32, 1.0)

            if cfg["parred"]:
                acc = [
                    accp.tile([128, M], f16, tag=f"acc{n}", name=f"acc{n}")
                    for n in range(N_BATCH)
                ]
            else:
                acc = [
                    accp.tile([128, 8], f32, tag=f"acc{n}", name=f"acc{n}")
                    for n in range(N_BATCH)
                ]
            for a in acc:
                nc.vector.memset(a, 0.0)

            for g in range(NGROUP):
                npair = _pairs_in_group(g)

                xg = io.tile([128, M], f32, tag="xg")
                nc.sync.dma_start(xg, X[g])
                yg = io.tile([128, M], f32, tag="yg")
                nc.sync.dma_start(yg, Y[g])

                # ---- group prep: mean, centering, channel norms ----
                ysum = tiny.tile([128, 1], f32, tag="ysum")
                if cfg["ysum_act"]:
                    yscr = sb.tile([128, M], f16, tag="yscr")
                    nc.scalar.activation(yscr, yg, AF.Copy, accum_out=ysum)
                elif cfg["ysum_pool"]:
                    nc.gpsimd.reduce_sum(ysum, yg, axis=AX)
                else:
                    nc.vector.reduce_sum(ysum, yg, axis=AX)
                mu_ps = psS.tile([128, 1], f32, tag="psS")
                nc.tensor.matmul(mu_ps, wmu_sb, ysum)
                mu = tiny.tile([128, 1], f32, tag="mu")
                nc.vector.tensor_copy(mu, mu_ps)

                ceng = nc.gpsimd if cfg["center_pool"] else nc.vector
                xc = sb.tile([128, M], f16, tag="xc")
                ceng.tensor_scalar(xc, xg, mu, None, op0=OP.subtract)
                yc = sb.tile([128, M], f16, tag="yc")
                ceng.tensor_scalar(yc, yg, mu, None, op0=OP.subtract)

                xsq = sb.tile([128, M], f16, tag="xsq")
                nc.gpsimd.tensor_mul(xsq, xc, xc)
                ysq = sb.tile([128, M], f16, tag="ysq")
                nc.gpsimd.tensor_mul(ysq, yc, yc)

                sy_ps = psS.tile([4, M], f32, tag="psS")
                nc.tensor.matmul(sy_ps, bd_sb, ysq)

                invxT = tiny.tile([128, 4, 4], f32, tag="invxT")
                if cfg["xnorm_direct"]:
                    # x norm^2 straight into the per-partition [i', c, q]
                    # layout: 16 single-column matmuls against a ones vector
                    sqx_ps = psS.tile([128, 4, 4], f32, tag="psS")
                    for q in range(4):
                        for c in range(4):
                            nc.tensor.matmul(
                                sqx_ps[:, c, q : q + 1],
                                xsq[32 * q : 32 * q + 32,
                                    128 * c : 128 * (c + 1)],
                                ones32[32 * q : 32 * q + 32, :],
                                tile_position=(32 * q, 0) if q else None,
                            )
                    lx = tiny.tile([128, 16], f32, tag="lx")
                    nc.scalar.activation(
                        lx, sqx_ps.rearrange("p a b -> p (a b)"),
                        AF.Ln, bias=c24,
                    )
                    nc.scalar.activation(
                        invxT.rearrange("p a b -> p (a b)"), lx,
                        AF.Exp, scale=-0.5,
                    )
                    # rinv carries only the y norms now
                    ls = sb.tile([4, 1, M], f32, tag="ls")
                    nc.scalar.activation(ls[:, 0, :], sy_ps, AF.Ln, bias=c24[:4])
                    rinv = sb.tile([4, 1, M], f16, tag="rinv")
                    nc.scalar.activation(rinv, ls, AF.Exp, scale=-0.5)
                    rny_src = rinv[:, 0, :]
                else:
                    sx_ps = psS.tile([4, M], f32, tag="psS")
                    nc.tensor.matmul(sx_ps, bd_sb, xsq)
                    # rinv = (S + floor)^-0.5 via log/exp (same ACT table set)
                    ls = sb.tile([4, 2, M], f32, tag="ls")
                    nc.scalar.activation(ls[:, 0, :], sx_ps, AF.Ln, bias=c24[:4])
                    nc.scalar.activation(ls[:, 1, :], sy_ps, AF.Ln, bias=c24[:4])
                    rinv = sb.tile([4, 2, M], f16, tag="rinv")
                    nc.scalar.activation(rinv, ls, AF.Exp, scale=-0.5)
                    rny_src = rinv[:, 1, :]

                # broadcast y-norms to all 4 slot blocks; yn = yc * rinv_y
                rny_ps = psS.tile([128, M], f32, tag="psS")
                nc.tensor.matmul(rny_ps, bdt_sb, rny_src)
                yn = sb.tile([128, M], f16, tag="yn")
                nc.vector.tensor_mul(yn, yc, rny_ps)

                if not cfg["xnorm_direct"]:
                    # x-norms transposed to [i', c, q] via tiny PE transposes
                    invxT_ps = psS.tile([128, 4, 4], f16, tag="psS")
                    for c in range(4):
                        nc.tensor.transpose(
                            invxT_ps[:, c, :],
                            rinv[:, 0, 128 * c : 128 * (c + 1)], id4_sb,
                        )
                    nc.vector.tensor_copy(invxT, invxT_ps)

                if not cfg["parred"] and cfg.get("ablate", 0) < 3:
                    colmax = tiny.tile([128, 4, 4], f32, tag="colmax")

                # ---- per (n, p) pair ----
                for q in range(npair):
                    lo = 32 * q
                    tp = (lo, 0) if lo else None

                    w = wpool.tile([128, 4, M], f16, tag="w")
                    mx4 = tiny.tile([128, 4], f32, tag="mx4")
                    rowsum = tiny.tile([128, 4], f32, tag="rowsum")
                    scale4 = tiny.tile([128, 4], f32, tag="scale4")
                    bias4 = tiny.tile([128, 4], f32, tag="bias4")
                    q4 = tiny.tile([128, 4], f32, tag="q4")
                    cm4 = tiny.tile([128, 4], f32, tag="cm4")
                    d4 = tiny.tile([128, 4], f32, tag="d4")
                    r4 = tiny.tile([128, 4], f32, tag="r4")
                    macc = wpool.tile([128, M], f16, tag="macc")
                    for h in range(2):
                        sl = slice(2 * h, 2 * h + 2)
                        gh = psA.tile([128, 2, M], f32, tag="G")
                        for cc in range(2):
                            c = 2 * h + cc
                            nc.tensor.matmul(
                                gh[:, cc, :],
                                xc[lo : lo + 32, 128 * c : 128 * (c + 1)],
                                yn[lo : lo + 32, :],
                                tile_position=tp,
                            )
                        abl = cfg.get("ablate", 0)
                        if abl < 2:
                            nc.vector.reduce_max(mx4[:, sl], gh, axis=AX)
                            # scale_i = invx/d, bias_i = 1-1/d,
                            # d = 1+eps-invx*mx
                            sme = (nc.gpsimd if cfg["smalls_eng"] == "pool"
                                   else nc.vector)
                            sme.tensor_mul(
                                cm4[:, sl], mx4[:, sl], invxT[:, sl, q]
                            )
                            sme.tensor_scalar(
                                d4[:, sl], cm4[:, sl], -1.0, 1.0 + EPS,
                                op0=OP.mult, op1=OP.add,
                            )
                            nc.vector.reciprocal(q4[:, sl], d4[:, sl])
                            sme.tensor_mul(
                                scale4[:, sl], q4[:, sl], invxT[:, sl, q]
                            )
                            sme.tensor_scalar(
                                bias4[:, sl], q4[:, sl], -1.0, 1.0,
                                op0=OP.mult, op1=OP.add,
                            )
                        if abl < 4:
                            for cc in range(2):
                                c = 2 * h + cc
                                kw = (dict(bias=bias4[:, c : c + 1],
                                           scale=scale4[:, c : c + 1])
                                      if abl < 2 else dict(scale=0.01))
                                nc.scalar.activation(
                                    w[:, c, :],
                                    gh[:, cc, :],
                                    AF.Exp,
                                    accum_out=rowsum[:, c : c + 1],
                                    **kw,
                                )
                            nc.vector.reciprocal(r4[:, sl], rowsum[:, sl])
                        if abl >= 3:
                            nc.vector.tensor_add(
                                acc[q % 2][:, 0:4], acc[q % 2][:, 0:4],
                                gh[:, 0, 0:4],
                            )
                            continue
                        # column-max accumulate: macc = max_c (w_c * r_c),
                        # streamed per half (half 1's mults on Pool)
                        if h == 0:
                            if cfg.get("macc0_pool"):
                                tmp0 = wpool.tile([128, M], f16, tag="tmp0")
                                nc.gpsimd.tensor_scalar(
                                    tmp0, w[:, 0, :], r4[:, 0:1], None,
                                    op0=OP.mult,
                                )
                                nc.vector.scalar_tensor_tensor(
                                    macc, w[:, 1, :], r4[:, 1:2], tmp0,
                                    op0=OP.mult, op1=OP.max,
                                )
                            else:
                                nc.vector.tensor_scalar(
                                    macc, w[:, 0, :], r4[:, 0:1], None,
                                    op0=OP.mult,
                                )
                                nc.vector.scalar_tensor_tensor(
                                    macc, w[:, 1, :], r4[:, 1:2], macc,
                                    op0=OP.mult, op1=OP.max,
                                )
                        elif cfg["maccmul_eng"] == "pool":
                            tmp2 = wpool.tile([128, M], f16, tag="tmp2")
                            nc.gpsimd.tensor_scalar(
                                tmp2, w[:, 2, :], r4[:, 2:3], None, op0=OP.mult
                            )
                            tmp3 = wpool.tile([128, M], f16, tag="tmp3")
                            nc.gpsimd.tensor_scalar(
                                tmp3, w[:, 3, :], r4[:, 3:4], None, op0=OP.mult
                            )
                            m23 = wpool.tile([128, M], f16, tag="m23")
                            nc.vector.tensor_tensor(m23, tmp2, tmp3, op=OP.max)
                            nc.vector.tensor_tensor(macc, macc, m23, op=OP.max)
                        else:
                            for cc in range(2):
                                c = 2 + cc
                                nc.vector.scalar_tensor_tensor(
                                    macc, w[:, c, :], r4[:, c : c + 1], macc,
                                    op0=OP.mult, op1=OP.max,
                                )

                    if cfg.get("ablate", 0) >= 3:
                        pass
                    elif cfg["parred"]:
                        # column max over all 512 i at once: Pool all-reduce
                        # across partitions (result replicated on 128 rows)
                        par = wpool.tile([128, M], f16, tag="par")
                        nc.gpsimd.partition_all_reduce(
                            par, macc, channels=128,
                            reduce_op=bass_isa.ReduceOp.max,
                        )
                        if cfg["acc_pool"]:
                            nc.gpsimd.tensor_add(acc[q % 2], acc[q % 2], par)
                        else:
                            nc.vector.tensor_add(acc[q % 2], acc[q % 2], par)
                    else:
                        t_ps = psT.tile([128, 4, 128], f16, tag="psT")
                        for t in range(4):
                            nc.tensor.transpose(
                                t_ps[:, t, :], macc[:, 128 * t : 128 * (t + 1)],
                                id128_sb,
                            )
                        nc.vector.reduce_max(colmax[:, q, :], t_ps, axis=AX)

                # accumulate per-batch: pair q has n = q % 2, sub-slot q // 2
                if not cfg["parred"] and cfg.get("ablate", 0) < 3:
                    if npair == 4:
                        for n in range(2):
                            nc.vector.tensor_add(
                                acc[n].rearrange("p (s t) -> p s t", s=2),
                                acc[n].rearrange("p (s t) -> p s t", s=2),
                                colmax[:, n::2, :],
                            )
                    else:
                        for n in range(2):
                            nc.vector.tensor_add(
                                acc[n][:, 0:4], acc[n][:, 0:4], colmax[:, n, :]
                            )

            for n in range(N_BATCH):
                if cfg["parred"]:
                    nc.sync.dma_start(OUT[n], acc[n][0:1, :])
                else:
                    nc.sync.dma_start(OUT[n], acc[n])

    _split_multiwaits(nc)
    return nc


def _to_patches(v):
    n, c, h, w, d = v.shape
    p = PATCH
    v = v.reshape(n, c, h // p, p, w // p, p, d // p, p)
    v = v.transpose(0, 2, 4, 6, 1, 3, 5, 7)
    return np.ascontiguousarray(v.reshape(n, -1, c, p**3))


def _pack_core(vp, k):
    sl = vp[:, PPC * k : PPC * (k + 1)]
    pad = np.zeros((N_BATCH, 2 * NGROUP - PPC, C, M), np.float32)
    arr = np.concatenate([sl, pad], axis=1)
    arr = arr.reshape(N_BATCH, NGROUP, 2, C, M)
    arr = arr.transpose(1, 2, 0, 3, 4)
    return np.ascontiguousarray(arr.reshape(NGROUP, 128, M))


def _consts():
    kk, pp = np.meshgrid(np.arange(128), np.arange(128), indexing="ij")
    wmu = np.where(
        (kk % 32 == pp % 32) & (kk // 64 == pp // 64), 1.0 / (N_BATCH * M), 0.0
    ).astype(np.float32)
    bd = np.zeros((128, 4), np.float16)
    bd[np.arange(128), np.arange(128) // 32] = 1.0
    bdt = np.ascontiguousarray(bd.T)
    id4 = np.eye(4, dtype=np.float16)
    id128 = np.eye(128, dtype=np.float16)
    return dict(wmu=wmu, bd=bd, bdt=bdt, id4=id4, id128=id128)


def kernel(x, y):
    global _BUILT
    x = np.ascontiguousarray(np.asarray(x), dtype=np.float32)
    y = np.ascontiguousarray(np.asarray(y), dtype=np.float32)
    xp = _to_patches(x)
    yp = _to_patches(y)

    if _BUILT is None:
        _BUILT = _build_module()
    nc = _BUILT

    consts = _consts()
    in_maps = [
        dict(xs=_pack_core(xp, k), ys=_pack_core(yp, k), **consts)
        for k in range(NCORES)
    ]
    res = run_bass_kernel_spmd(nc, in_maps, core_ids=list(range(NCORES)))

    if DEFAULT_CFG["parred"]:
        tot = np.zeros((N_BATCH, M), np.float64)
        for r in res.results:
            tot += r["acc_out"].astype(np.float64).reshape(N_BATCH, M)
        cx_tot = tot / P_TOT
    else:
        tot = np.zeros((N_BATCH, 128, 8), np.float64)
        for r in res.results:
            tot += r["acc_out"].astype(np.float64)
        tot4 = tot.reshape(N_BATCH, 128, 2, 4).sum(axis=2)
        cx_tot = tot4.transpose(0, 2, 1).reshape(N_BATCH, M) / P_TOT
    loss = np.mean(-np.log(cx_tot + EPS))
    return np.float32(loss)
